# revision 7
# baseline (speedup 1.0000x reference)
"""Trainium2 Bass kernel for nn_DependencyBertMix.

Contract: kernel(**inputs) takes the FULL unsharded inputs (as produced by
setup_inputs()) and returns the FULL [8, 512, 768] float32 output.

Strategy: data-parallel over batch - B=8 batch elements, one per NeuronCore.
Weights are replicated to all 8 cores; no collectives.

Per-core pipeline in transposed [feature, t] layout (t = query, s = key):

  Q'_T = (Wq/8)^T @ hid_T   (scores pre-scaled via Wq)   K_T, V likewise
  per head h:
    A_T[s,t]  = K_h as lhsT @ Q'_h          (= self_attn^T, bf16 copy)
    D_T       = A (.) dep^T                  (dep_self_attn^T)
    diff'     = A (.) 0.5(1-dep)^T           (= (A-D)/2)
    s1        = D + diff'                    (mix base)
    stats     : mu = (ksum_h @ Q'_h + colsum(D)) / 2T   (ksum = rowsum K_T)
                ms = colsum(A^2 (.) (1+dep^2)^T) = colsum(A^2+D^2)
                var/rsqrt on-chip, replicated rows via ones-matmuls
    LayerNorm + gating MLP folded into matmuls:
      Y[n,t]  = Wg^T @ [A;D] - s_vec x mu   (+ c_vec x sqrt(var+eps))
      th      = tanh(Y * rs)
      t2      = tanh(0.5 (W2^T th + b2))     (sigmoid(x) = 0.5+0.5 tanh(x/2))
    mixd      = s1 + t2 (.) diff'  =  g*A + (1-g)*D
    E = exp(mixd);  ctx[t,d] = E^T-chunks @ [V_h|1], * 1/den -> [T,C] output

Emission is a depth-2 software pipeline (iteration i runs scores/stats of
pair i, ctx of pair i-2, and the gating MLP of pair i-1) so the tensor
engine always has ready matmuls and PSUM slots recycle without stalls.
"""

import sys

for _p in ("/opt/trn_rl_repo", "/opt/pypackages"):
    if _p not in sys.path:
        sys.path.append(_p)

import ml_dtypes
import numpy as np

B, T, C = 8, 512, 768
H, DH = 12, 64
TM = 512
EPS = 1e-5
N_CORES = 8
P = 128


def _build(flags):
    import concourse.tile as tile
    from concourse import bacc, mybir

    f32 = mybir.dt.float32
    f32r = mybir.dt.float32r
    bf16 = mybir.dt.bfloat16
    AF = mybir.ActivationFunctionType
    OP = mybir.AluOpType
    AX = mybir.AxisListType

    nc = bacc.Bacc("TRN2", target_bir_lowering=False, debug=False,
                   enable_asserts=False, num_devices=N_CORES)

    masked = flags["mask"]

    # ---- DRAM I/O (host-prepared layouts; weights pre-cast to bf16) ----
    hid_t = nc.dram_tensor("hid_t", [C, T], bf16, kind="ExternalInput")
    dep_t = nc.dram_tensor("dep_t", [T, T], bf16, kind="ExternalInput")  # dep^T
    dpm_t = nc.dram_tensor("dpm_t", [T, T], bf16, kind="ExternalInput")  # (1-dep)^T/2
    if not masked:
        u_t = nc.dram_tensor("u_t", [T, T], bf16, kind="ExternalInput")  # 1+dep^2
    wq = nc.dram_tensor("wq", [C, C], bf16, kind="ExternalInput")  # pre /8
    wk = nc.dram_tensor("wk", [C, C], bf16, kind="ExternalInput")
    wv = nc.dram_tensor("wv", [C, C], bf16, kind="ExternalInput")
    w1 = nc.dram_tensor("w1", [2 * TM, TM], bf16, kind="ExternalInput")
    w2 = nc.dram_tensor("w2", [TM, TM], bf16, kind="ExternalInput")
    out_t = nc.dram_tensor("out_t", [T, C], f32, kind="ExternalOutput")

    bq_d = nc.dram_tensor("bq", [C], f32, kind="ExternalInput") if flags["bq"] else None
    bk_d = nc.dram_tensor("bk", [C], f32, kind="ExternalInput") if flags["bk"] else None
    bv_d = (nc.dram_tensor("bv", [C], bf16, kind="ExternalInput")
            if flags["bv"] else None)
    lng_d = (nc.dram_tensor("lng", [2 * TM], f32, kind="ExternalInput")
             if flags["lng"] else None)
    if flags["c"]:
        lnb_d = nc.dram_tensor("lnb", [2 * TM], bf16, kind="ExternalInput")
        b1_d = nc.dram_tensor("b1", [TM], f32, kind="ExternalInput")
    b2_d = (nc.dram_tensor("b2", [TM], bf16, kind="ExternalInput")
            if flags["b2"] else None)
    mb_d = (nc.dram_tensor("mb", [T], f32, kind="ExternalInput")
            if masked else None)

    CI = C // P   # 6
    CO = C // P   # 6
    ST = T // P   # 4
    TT = T // P   # 4
    KT8 = 2 * TM // P  # 8
    NT = TM // P  # 4

    with tile.TileContext(nc) as tc:
        with (
            tc.tile_pool(name="singles", bufs=1) as singles,
            tc.tile_pool(name="wpool", bufs=3) as wpool,
            tc.tile_pool(name="adpool", bufs=34) as adpool,
            tc.tile_pool(name="dfpool", bufs=17) as dfpool,
            tc.tile_pool(name="s1pool", bufs=17) as s1pool,
            tc.tile_pool(name="sqpool", bufs=10) as sqpool,
            tc.tile_pool(name="stpool", bufs=8) as stpool,
            tc.tile_pool(name="vrpool", bufs=6) as vrpool,
            tc.tile_pool(name="tipool", bufs=6) as tipool,
            tc.tile_pool(name="thpool", bufs=8) as thpool,
            tc.tile_pool(name="t2pool", bufs=8) as t2pool,
            tc.tile_pool(name="mixpool", bufs=6) as mixpool,
            tc.tile_pool(name="epool", bufs=16) as epool,
            tc.tile_pool(name="rdpool", bufs=4) as rdpool,
            tc.tile_pool(name="opool", bufs=4) as opool,
            tc.tile_pool(name="pspool", bufs=8, space="PSUM") as pspool,
        ):
            def ps_tile():
                return pspool.tile([P, 512], f32, tag="ps", name="ps")

            # ---------- early DMAs: only what QKV needs ----------
            hid_l = [adpool.tile([P, T], bf16, tag="ad", name=f"hid{ci}")
                     for ci in range(CI)]
            for ci in range(CI):
                nc.sync.dma_start(out=hid_l[ci][:],
                                  in_=hid_t[ci * P:(ci + 1) * P, :])

            ones_b = singles.tile([P, P], bf16)
            nc.vector.memset(ones_b[:], 1.0)
            eps_col = singles.tile([P, 1], f32)
            nc.vector.memset(eps_col[:], EPS)

            bq_sb = None
            if flags["bq"]:
                bq_sb = singles.tile([P, CO], f32)
                nc.sync.dma_start(out=bq_sb[:],
                                  in_=bq_d[:].rearrange("(j p) -> p j", p=P))
            bk_sb = None
            if flags["bk"]:
                bk_sb = singles.tile([P, CO], f32)
                nc.sync.dma_start(out=bk_sb[:],
                                  in_=bk_d[:].rearrange("(j p) -> p j", p=P))
            bv_sb = None
            if flags["bv"]:
                bv_sb = singles.tile([1, C], bf16)
                nc.sync.dma_start(out=bv_sb[:], in_=bv_d[None, :])
            mb_sb = None
            if masked:
                mb_sb = singles.tile([P, ST], f32)
                nc.sync.dma_start(out=mb_sb[:],
                                  in_=mb_d[:].rearrange("(j p) -> p j", p=P))

            # ---------- QKV projections ----------
            QT = [singles.tile([P, T], bf16, tag=f"qt{i}", name=f"qt{i}")
                  for i in range(CO)]
            KTt = [singles.tile([P, T], bf16, tag=f"kt{i}", name=f"kt{i}")
                   for i in range(CO)]
            vaug = [singles.tile([P, H, DH + 1], bf16, tag=f"v{i}", name=f"v{i}")
                    for i in range(TT)]
            for tt in range(TT):
                nc.vector.memset(vaug[tt][:, :, DH:DH + 1], 1.0)

            for wdram, dest, bsb in ((wq, QT, bq_sb), (wk, KTt, bk_sb)):
                ps_l = [ps_tile() for _ in range(CO)]
                for ci in range(CI):
                    w_ci = wpool.tile([P, C], bf16, tag="w", name="w")
                    nc.sync.dma_start(out=w_ci[:], in_=wdram[ci * P:(ci + 1) * P, :])
                    for cot in range(CO):
                        nc.tensor.matmul(ps_l[cot][:],
                                         lhsT=w_ci[:, cot * P:(cot + 1) * P],
                                         rhs=hid_l[ci][:],
                                         start=(ci == 0), stop=(ci == CI - 1))
                for cot in range(CO):
                    if bsb is not None:
                        nc.scalar.activation(dest[cot][:], ps_l[cot][:], AF.Identity,
                                             bias=bsb[:, cot:cot + 1])
                    else:
                        nc.scalar.copy(dest[cot][:], ps_l[cot][:])

            # rowsums of K_T per cotile (for the A-half of the mean)
            ksum_sb = None
            ksr = None
            if not masked:
                ksum_sb = singles.tile([P, CO], f32)
                ksr = [singles.tile([P, P], bf16, tag=f"ksr{i}", name=f"ksr{i}")
                       for i in range(CO)]
                for cot in range(CO):
                    nc.vector.tensor_reduce(ksum_sb[:, cot:cot + 1], KTt[cot][:],
                                            axis=AX.X, op=OP.add)
                    nc.vector.tensor_scalar_mul(ksr[cot][:], ones_b[:],
                                                ksum_sb[:, cot:cot + 1])

            NCH = 2
            CHW = C // NCH  # 384
            v_ps = [[pspool.tile([P, CHW], f32, tag="ps", name="vps")
                     for _ in range(NCH)] for _ in range(TT)]
            last_v = CI - 1 if not flags["bv"] else None
            for ci in range(CI):
                w_ci = wpool.tile([P, C], bf16, tag="w", name="w")
                nc.sync.dma_start(out=w_ci[:], in_=wv[ci * P:(ci + 1) * P, :])
                for tt in range(TT):
                    for ch in range(NCH):
                        nc.tensor.matmul(
                            v_ps[tt][ch][:],
                            lhsT=hid_l[ci][:, tt * P:(tt + 1) * P],
                            rhs=w_ci[:, ch * CHW:(ch + 1) * CHW],
                            start=(ci == 0), stop=(ci == last_v))
            HPC = CHW // DH  # 6 heads per chunk
            for tt in range(TT):
                for ch in range(NCH):
                    if flags["bv"]:
                        nc.tensor.matmul(v_ps[tt][ch][:],
                                         lhsT=ones_b[0:1, :],
                                         rhs=bv_sb[:, ch * CHW:(ch + 1) * CHW],
                                         start=False, stop=True)
                    for hh in range(HPC):
                        nc.scalar.copy(vaug[tt][:, ch * HPC + hh, 0:DH],
                                       v_ps[tt][ch][:, hh * DH:(hh + 1) * DH])

            # ---------- late DMAs: dep maps + gating weights ----------
            dep_sb = singles.tile([P, ST, T], bf16)
            for j in range(ST):
                nc.sync.dma_start(out=dep_sb[:, j, :],
                                  in_=dep_t[j * P:(j + 1) * P, :])
            dpm_sb = singles.tile([P, ST, T], bf16)
            for j in range(ST):
                nc.sync.dma_start(out=dpm_sb[:, j, :],
                                  in_=dpm_t[j * P:(j + 1) * P, :])
            u_sb = None
            if not masked:
                u_sb = singles.tile([P, ST, T], bf16)
                for j in range(ST):
                    nc.sync.dma_start(out=u_sb[:, j, :],
                                      in_=u_t[j * P:(j + 1) * P, :])

            wg_sb = singles.tile([P, KT8, TM], bf16)
            for j in range(KT8):
                nc.sync.dma_start(out=wg_sb[:, j, :], in_=w1[j * P:(j + 1) * P, :])
            w2_sb = singles.tile([P, NT, TM], bf16)
            for j in range(NT):
                nc.sync.dma_start(out=w2_sb[:, j, :], in_=w2[j * P:(j + 1) * P, :])

            c_sb = None
            if flags["c"]:
                lnb_sb = singles.tile([P, KT8], bf16)
                nc.sync.dma_start(out=lnb_sb[:],
                                  in_=lnb_d[:].rearrange("(j p) -> p j", p=P))
                b1_sb = singles.tile([1, TM], f32)
                nc.sync.dma_start(out=b1_sb[:], in_=b1_d[None, :])
                c_ps = ps_tile()
                for j in range(KT8):
                    nc.tensor.matmul(c_ps[0:1, :], lhsT=lnb_sb[:, j:j + 1],
                                     rhs=wg_sb[:, j, :],
                                     start=(j == 0), stop=(j == KT8 - 1))
                c_sb = singles.tile([1, TM], f32r)
                nc.vector.tensor_add(c_sb[:], c_ps[0:1, :], b1_sb[:])

            if flags["lng"]:
                lng_sb = singles.tile([P, KT8], f32)
                nc.sync.dma_start(out=lng_sb[:],
                                  in_=lng_d[:].rearrange("(j p) -> p j", p=P))
                for j in range(KT8):
                    nc.vector.tensor_scalar_mul(wg_sb[:, j, :], wg_sb[:, j, :],
                                                lng_sb[:, j:j + 1])

            # s_vec = column sums of Wg (post ln_g fold); store negated f32r row
            s_ps = ps_tile()
            for j in range(KT8):
                nc.tensor.matmul(s_ps[0:1, :], lhsT=ones_b[:, 0:1],
                                 rhs=wg_sb[:, j, :],
                                 start=(j == 0), stop=(j == KT8 - 1))
            negs_sb = singles.tile([1, TM], f32r)
            nc.scalar.activation(negs_sb[:], s_ps[0:1, :], AF.Copy, scale=-1.0)

            b2_sb = None
            if flags["b2"]:
                b2_sb = singles.tile([1, TM], bf16)
                nc.sync.dma_start(out=b2_sb[:], in_=b2_d[None, :])

            # ---------- per-head pipeline ----------
            inv2t = 1.0 / (2 * TM)

            def scores_phase(h):
                """scores matmuls + A copy + D/diff'/s1 + squares for ms."""
                cot, poff = h // 2, (h % 2) * DH
                q_h = QT[cot][poff:poff + DH, :]
                k_h = KTt[cot][poff:poff + DH, :]
                A = [adpool.tile([P, T], bf16, tag="ad", name="A")
                     for _ in range(ST)]
                D = [adpool.tile([P, T], bf16, tag="ad", name="D")
                     for _ in range(ST)]
                dfs, s1s, wsqs = [], [], []
                for j in range(ST):
                    sp = ps_tile()
                    nc.tensor.matmul(sp[:], lhsT=k_h[:, j * P:(j + 1) * P],
                                     rhs=q_h, start=True, stop=True)
                    df = dfpool.tile([P, T], bf16, tag="df", name="df")
                    s1 = s1pool.tile([P, T], bf16, tag="s1", name="s1")
                    if masked:
                        nc.scalar.activation(A[j][:], sp[:], AF.Identity,
                                             bias=mb_sb[:, j:j + 1])
                        nc.vector.tensor_mul(D[j][:], sp[:], dep_sb[:, j, :])
                        # df = 0.5*(A - D) (mask bias lives in A only)
                        nc.vector.tensor_sub(df[:], A[j][:], D[j][:])
                        nc.vector.tensor_scalar_mul(df[:], df[:], 0.5)
                        nc.vector.tensor_add(s1[:], D[j][:], df[:])
                        sqa = sqpool.tile([P, T], bf16, tag="sq", name="sqa")
                        nc.vector.tensor_mul(sqa[:], A[j][:], A[j][:])
                        sqd = sqpool.tile([P, T], bf16, tag="sq", name="sqd")
                        nc.vector.tensor_mul(sqd[:], D[j][:], D[j][:])
                        wsqs.append((sqa, sqd))
                    else:
                        nc.scalar.copy(A[j][:], sp[:])
                        nc.vector.tensor_mul(D[j][:], A[j][:], dep_sb[:, j, :])
                        nc.vector.tensor_mul(df[:], A[j][:], dpm_sb[:, j, :])
                        nc.vector.tensor_add(s1[:], D[j][:], df[:])
                        sq = sqpool.tile([P, T], bf16, tag="sq", name="sq")
                        nc.vector.tensor_mul(sq[:], A[j][:], A[j][:])
                        wsq = sqpool.tile([P, T], bf16, tag="sq", name="wsq")
                        nc.vector.tensor_mul(wsq[:], sq[:], u_sb[:, j, :])
                        wsqs.append(wsq)
                    dfs.append(df)
                    s1s.append(s1)
                return A, D, dfs, s1s, wsqs

            def stats_mm_phase(h, A, D, wsqs):
                cot, poff = h // 2, (h % 2) * DH
                mu_ps = ps_tile()
                ms_ps = ps_tile()
                if masked:
                    for j, src in enumerate(A + D):
                        nc.tensor.matmul(mu_ps[:], lhsT=ones_b[:], rhs=src[:],
                                         start=(j == 0), stop=(j == 2 * ST - 1))
                    flat = [t for pair in wsqs for t in pair]
                    for j, src in enumerate(flat):
                        nc.tensor.matmul(ms_ps[:], lhsT=ones_b[:], rhs=src[:],
                                         start=(j == 0), stop=(j == len(flat) - 1))
                else:
                    nc.tensor.matmul(mu_ps[:],
                                     lhsT=ksr[cot][poff:poff + DH, :],
                                     rhs=QT[cot][poff:poff + DH, :],
                                     start=True, stop=False)
                    for j in range(ST):
                        nc.tensor.matmul(mu_ps[:], lhsT=ones_b[:], rhs=D[j][:],
                                         start=False, stop=(j == ST - 1))
                    for j, src in enumerate(wsqs):
                        nc.tensor.matmul(ms_ps[:], lhsT=ones_b[:], rhs=src[:],
                                         start=(j == 0), stop=(j == ST - 1))
                return mu_ps, ms_ps

            def stats_fin_phase(mu_ps, ms_ps):
                mu_rep = stpool.tile([P, T], f32r, tag="st", name="mu_rep")
                nc.scalar.activation(mu_rep[:], mu_ps[:], AF.Copy, scale=inv2t)
                mu2 = vrpool.tile([P, T], f32, tag="vr", name="mu2")
                nc.vector.tensor_mul(mu2[:], mu_rep[:], mu_rep[:])
                var = vrpool.tile([P, T], f32, tag="vr", name="var")
                nc.vector.scalar_tensor_tensor(var[:], ms_ps[:], inv2t, mu2[:],
                                               op0=OP.mult, op1=OP.subtract)
                pool = stpool if flags["c"] else vrpool
                tag = "st" if flags["c"] else "vr"
                sqv_rep = pool.tile([P, T], f32r, tag=tag, name="sqv_rep")
                nc.scalar.activation(sqv_rep[:], var[:], AF.Sqrt, bias=eps_col[:])
                rs_rep = stpool.tile([P, T], f32, tag="st", name="rs_rep")
                nc.vector.reciprocal(rs_rep[:], sqv_rep[:])
                return mu_rep, sqv_rep, rs_rep

            def mlp1_phase(A, D, mu_rep, sqv_rep, rs_rep):
                ti_l = []
                for nt in range(NT):
                    y_ps = ps_tile()
                    nsl = slice(nt * P, (nt + 1) * P)
                    for j, src in enumerate(A + D):
                        nc.tensor.matmul(y_ps[:], lhsT=wg_sb[:, j, nsl],
                                         rhs=src[:], start=(j == 0), stop=False)
                    nc.tensor.matmul(y_ps[:], lhsT=negs_sb[0:1, nsl],
                                     rhs=mu_rep[0:1, :],
                                     start=False, stop=not flags["c"])
                    if flags["c"]:
                        nc.tensor.matmul(y_ps[:], lhsT=c_sb[0:1, nsl],
                                         rhs=sqv_rep[0:1, :],
                                         start=False, stop=True)
                    ti = tipool.tile([P, T], bf16, tag="ti", name="ti")
                    nc.vector.tensor_mul(ti[:], y_ps[:], rs_rep[:])
                    ti_l.append(ti)
                return ti_l

            def tanh_phase(ti_l):
                th_l = []
                for ti in ti_l:
                    th = thpool.tile([P, T], bf16, tag="th", name="th")
                    th_l.append(th)
                    nc.scalar.activation(th[:], ti[:], AF.Tanh)
                return th_l

            def mlp2_phase(th_l):
                g_ps_l = []
                for nt in range(NT):
                    g_ps = ps_tile()
                    nsl = slice(nt * P, (nt + 1) * P)
                    for j in range(NT):
                        nc.tensor.matmul(g_ps[:], lhsT=w2_sb[:, j, nsl],
                                         rhs=th_l[j][:], start=(j == 0),
                                         stop=(j == NT - 1 and not flags["b2"]))
                    if flags["b2"]:
                        nc.tensor.matmul(g_ps[:], lhsT=b2_sb[0:1, nsl],
                                         rhs=ones_b[0:1, :],
                                         start=False, stop=True)
                    g_ps_l.append(g_ps)
                return g_ps_l

            def tanh2_phase(g_ps_l):
                # sigmoid(x) = 0.5 + 0.5*tanh(x/2); the affine part is folded
                # into the mix: mixd = s1 + t2*diff'
                t2_l = []
                for g_ps in g_ps_l:
                    t2 = t2pool.tile([P, T], bf16, tag="t2", name="t2")
                    t2_l.append(t2)
                    nc.scalar.activation(t2[:], g_ps[:], AF.Tanh, scale=0.5)
                return t2_l

            def mix_exp_phase(dfs, s1s, t2_l):
                E_l = []
                for j in range(ST):
                    prod = mixpool.tile([P, T], bf16, tag="mix", name="prod")
                    nc.gpsimd.tensor_mul(prod[:], t2_l[j][:], dfs[j][:])
                    mixd = mixpool.tile([P, T], bf16, tag="mix", name="mixd")
                    nc.gpsimd.tensor_add(mixd[:], prod[:], s1s[j][:])
                    E = epool.tile([P, T], bf16, tag="e", name="E")
                    nc.scalar.activation(E[:], mixd[:], AF.Exp)
                    E_l.append(E)
                return E_l

            def ctx_phase(h, E_l):
                # one psum bank: [ctx|den] chunks of width 65 at cols c*65
                W65 = DH + 1
                dc = ps_tile()
                for cch in range(4):
                    for j in range(ST):
                        nc.tensor.matmul(
                            dc[:, cch * W65:(cch + 1) * W65],
                            lhsT=E_l[j][:, cch * P:(cch + 1) * P],
                            rhs=vaug[j][:, h, :],
                            start=(j == 0), stop=(j == ST - 1))
                rden = rdpool.tile([P, 4], f32, tag="rd", name="rden")
                nc.vector.reciprocal(rden[:], dc[:, DH:4 * W65:W65])
                ctxn = opool.tile([P, 4, DH], f32, tag="o", name="ctxn")
                for cch in range(4):
                    nc.vector.tensor_scalar_mul(ctxn[:, cch, :],
                                                dc[:, cch * W65:cch * W65 + DH],
                                                rden[:, cch:cch + 1])
                nc.sync.dma_start(
                    out=out_t[:, h * DH:(h + 1) * DH].rearrange(
                        "(j p) c -> p j c", p=P),
                    in_=ctxn[:])

            # depth-2 software pipeline over head pairs
            NP = H // 2
            state = {}
            for pc in range(NP + 2):
                if pc < NP:
                    h0, h1 = 2 * pc, 2 * pc + 1
                    A0, D0, df0, s10, wsq0 = scores_phase(h0)
                    A1, D1, df1, s11, wsq1 = scores_phase(h1)
                    sm0 = stats_mm_phase(h0, A0, D0, wsq0)
                    sm1 = stats_mm_phase(h1, A1, D1, wsq1)
                    st0 = stats_fin_phase(*sm0)
                    st1 = stats_fin_phase(*sm1)
                if pc >= 2:
                    Ep = state[pc - 2]["E"]
                    ctx_phase(2 * (pc - 2), Ep[0])
                    ctx_phase(2 * (pc - 2) + 1, Ep[1])
                    del state[pc - 2]
                if 1 <= pc <= NP:
                    s = state[pc - 1]
                    ti0 = mlp1_phase(s["A0"], s["D0"], *s["st0"])
                    ti1 = mlp1_phase(s["A1"], s["D1"], *s["st1"])
                    th0 = tanh_phase(ti0)
                    th1 = tanh_phase(ti1)
                    gp0 = mlp2_phase(th0)
                    gp1 = mlp2_phase(th1)
                    t20 = tanh2_phase(gp0)
                    t21 = tanh2_phase(gp1)
                    e0 = mix_exp_phase(s["df0"], s["s10"], t20)
                    e1 = mix_exp_phase(s["df1"], s["s11"], t21)
                    s["E"] = (e0, e1)
                    # drop refs no longer needed
                    for k in ("A0", "A1", "D0", "D1", "df0", "df1",
                              "s10", "s11", "st0", "st1"):
                        s[k] = None
                if pc < NP:
                    state[pc] = dict(A0=A0, D0=D0, df0=df0, s10=s10, st0=st0,
                                     A1=A1, D1=D1, df1=df1, s11=s11, st1=st1)

    nc.compile()
    return nc


def _prep(inputs):
    bfloat16 = ml_dtypes.bfloat16
    hidden = np.asarray(inputs["hidden_states"], dtype=np.float32)
    mask = np.asarray(inputs["attention_mask"], dtype=np.float32)
    dep = np.asarray(inputs["dependency_matrix"], dtype=np.float32)
    ws = {k: np.asarray(inputs[k], dtype=np.float32)
          for k in ("Wq", "Wk", "Wv", "W1", "W2")}
    vs = {k: np.asarray(inputs[k], dtype=np.float32)
          for k in ("bq", "bk", "bv", "b1", "b2", "ln_g", "ln_b")}
    mb = (1.0 - mask) * -10000.0

    flags = {
        "bq": bool(np.any(vs["bq"])), "bk": bool(np.any(vs["bk"])),
        "bv": bool(np.any(vs["bv"])),
        "lng": bool(np.any(vs["ln_g"] != 1.0)),
        "c": bool(np.any(vs["ln_b"]) or np.any(vs["b1"])),
        "b2": bool(np.any(vs["b2"])),
        "mask": bool(np.any(mb)),
    }

    wq_b = np.ascontiguousarray((ws["Wq"] * np.float32(0.125)).astype(bfloat16))
    wk_b = np.ascontiguousarray(ws["Wk"].astype(bfloat16))
    wv_b = np.ascontiguousarray(ws["Wv"].astype(bfloat16))
    w1_b = np.ascontiguousarray(ws["W1"].astype(bfloat16))
    w2_b = np.ascontiguousarray(ws["W2"].astype(bfloat16))

    in_maps = []
    for b in range(N_CORES):
        dt = dep[b].T
        m = {
            "hid_t": np.ascontiguousarray(hidden[b].T.astype(bfloat16)),
            "dep_t": np.ascontiguousarray(dt.astype(bfloat16)),
            "dpm_t": np.ascontiguousarray((0.5 * (1.0 - dt)).astype(bfloat16)),
            "wq": wq_b, "wk": wk_b, "wv": wv_b,
            "w1": w1_b, "w2": w2_b,
        }
        if not flags["mask"]:
            m["u_t"] = np.ascontiguousarray((1.0 + dt * dt).astype(bfloat16))
        if flags["bq"]:
            m["bq"] = vs["bq"] * np.float32(0.125)
        if flags["bk"]:
            m["bk"] = vs["bk"]
        if flags["bv"]:
            m["bv"] = vs["bv"].astype(bfloat16)
        if flags["lng"]:
            m["lng"] = vs["ln_g"]
        if flags["c"]:
            m["lnb"] = vs["ln_b"].astype(bfloat16)
            m["b1"] = vs["b1"]
        if flags["b2"]:
            m["b2"] = vs["b2"].astype(bfloat16)
        if flags["mask"]:
            m["mb"] = np.ascontiguousarray(mb[b])
        in_maps.append(m)
    return flags, in_maps


def kernel(**inputs):
    from concourse.bass_utils import run_bass_kernel_spmd

    flags, in_maps = _prep(inputs)
    nc = _build(flags)
    res = run_bass_kernel_spmd(nc, in_maps, core_ids=list(range(N_CORES)))
    out = np.stack([r["out_t"] for r in res.results])
    return out.astype(np.float32)


# revision 12
# speedup vs baseline: 1.2222x; 1.2222x over previous
"""Trainium2 Bass kernel for nn_DependencyBertMix.

Contract: kernel(**inputs) takes the FULL unsharded inputs (as produced by
setup_inputs()) and returns the FULL [8, 512, 768] float32 output.

Strategy: data-parallel over batch - B=8 batch elements, one per NeuronCore.
Weights are replicated to all 8 cores; no collectives.

Per-core pipeline in transposed [feature, t] layout (t = query, s = key):

  Q'_T = (Wq/8)^T @ hid_T   (scores pre-scaled via Wq)   K_T, V likewise
  per head h:
    A_T[s,t]  = K_h as lhsT @ Q'_h          (= self_attn^T, bf16 copy)
    D_T       = A (.) dep^T                  (dep_self_attn^T)
    diff'     = A (.) 0.5(1-dep)^T           (= (A-D)/2)
    s1        = D + diff'                    (mix base)
    stats     : mu = (ksum_h @ Q'_h + colsum(D)) / 2T   (ksum = rowsum K_T)
                ms = colsum(A^2 (.) (1+dep^2)^T) = colsum(A^2+D^2)
                var/rsqrt on-chip, replicated rows via ones-matmuls
    LayerNorm + gating MLP folded into matmuls:
      Y[n,t]  = Wg^T @ [A;D] - s_vec x mu   (+ c_vec x sqrt(var+eps))
      th      = tanh(Y * rs)
      t2      = tanh(0.5 (W2^T th + b2))     (sigmoid(x) = 0.5+0.5 tanh(x/2))
    mixd      = s1 + t2 (.) diff'  =  g*A + (1-g)*D
    E = exp(mixd);  ctx[t,d] = E^T-chunks @ [V_h|1], * 1/den -> [T,C] output

Emission is a depth-2 software pipeline (iteration i runs scores/stats of
pair i, ctx of pair i-2, and the gating MLP of pair i-1) so the tensor
engine always has ready matmuls and PSUM slots recycle without stalls.
"""

import sys

for _p in ("/opt/trn_rl_repo", "/opt/pypackages"):
    if _p not in sys.path:
        sys.path.append(_p)

import ml_dtypes
import numpy as np

B, T, C = 8, 512, 768
H, DH = 12, 64
TM = 512
EPS = 1e-5
N_CORES = 8
P = 128


def _build(flags):
    import concourse.tile as tile
    from concourse import bacc, mybir

    f32 = mybir.dt.float32
    f32r = mybir.dt.float32r
    bf16 = mybir.dt.bfloat16
    AF = mybir.ActivationFunctionType
    OP = mybir.AluOpType
    AX = mybir.AxisListType

    nc = bacc.Bacc("TRN2", target_bir_lowering=False, debug=False,
                   enable_asserts=False, num_devices=N_CORES)

    masked = flags["mask"]

    # ---- DRAM I/O (host-prepared layouts; weights pre-cast to bf16) ----
    hid_t = nc.dram_tensor("hid_t", [C, T], bf16, kind="ExternalInput")
    dep_t = nc.dram_tensor("dep_t", [T, T], bf16, kind="ExternalInput")  # dep^T
    dpm_t = nc.dram_tensor("dpm_t", [T, T], bf16, kind="ExternalInput")  # (1-dep)^T/2
    if not masked:
        u_t = nc.dram_tensor("u_t", [T, T], bf16, kind="ExternalInput")  # 1+dep^2
    wq = nc.dram_tensor("wq", [C, C], bf16, kind="ExternalInput")  # pre /8
    wk = nc.dram_tensor("wk", [C, C], bf16, kind="ExternalInput")
    wv = nc.dram_tensor("wv", [C, C], bf16, kind="ExternalInput")
    w1 = nc.dram_tensor("w1", [2 * TM, TM], bf16, kind="ExternalInput")
    w2 = nc.dram_tensor("w2", [TM, TM], bf16, kind="ExternalInput")
    out_t = nc.dram_tensor("out_t", [T, C], f32, kind="ExternalOutput")

    bq_d = nc.dram_tensor("bq", [C], f32, kind="ExternalInput") if flags["bq"] else None
    bk_d = nc.dram_tensor("bk", [C], f32, kind="ExternalInput") if flags["bk"] else None
    bv_d = (nc.dram_tensor("bv", [C], bf16, kind="ExternalInput")
            if flags["bv"] else None)
    lng_d = (nc.dram_tensor("lng", [2 * TM], f32, kind="ExternalInput")
             if flags["lng"] else None)
    if flags["c"]:
        lnb_d = nc.dram_tensor("lnb", [2 * TM], bf16, kind="ExternalInput")
        b1_d = nc.dram_tensor("b1", [TM], f32, kind="ExternalInput")
    b2_d = (nc.dram_tensor("b2", [TM], bf16, kind="ExternalInput")
            if flags["b2"] else None)
    mb_d = (nc.dram_tensor("mb", [T], f32, kind="ExternalInput")
            if masked else None)

    CI = C // P   # 6
    CO = C // P   # 6
    ST = T // P   # 4
    TT = T // P   # 4
    KT8 = 2 * TM // P  # 8
    NT = TM // P  # 4

    with tile.TileContext(nc) as tc:
        with (
            tc.tile_pool(name="singles", bufs=1) as singles,
            tc.tile_pool(name="wpool", bufs=6) as wpool,
            tc.tile_pool(name="adpool", bufs=34) as adpool,
            tc.tile_pool(name="dfpool", bufs=17) as dfpool,
            tc.tile_pool(name="s1pool", bufs=17) as s1pool,
            tc.tile_pool(name="sqpool", bufs=10) as sqpool,
            tc.tile_pool(name="stpool", bufs=8) as stpool,
            tc.tile_pool(name="vrpool", bufs=6) as vrpool,
            tc.tile_pool(name="tipool", bufs=6) as tipool,
            tc.tile_pool(name="thpool", bufs=7) as thpool,
            tc.tile_pool(name="t2pool", bufs=7) as t2pool,
            tc.tile_pool(name="mixpool", bufs=6) as mixpool,
            tc.tile_pool(name="epool", bufs=15) as epool,
            tc.tile_pool(name="rdpool", bufs=4) as rdpool,
            tc.tile_pool(name="opool", bufs=4) as opool,
            tc.tile_pool(name="psf", bufs=4, space="PSUM") as psf,
            tc.tile_pool(name="psb", bufs=4, space="PSUM") as psb,
        ):
            def ps_tile(pool=None):
                pool = pool if pool is not None else psf
                return pool.tile([P, 512], f32, tag="ps", name="ps")

            # ---------- early DMAs: only what QKV needs ----------
            hid_l = [adpool.tile([P, T], bf16, tag="ad", name=f"hid{ci}")
                     for ci in range(CI)]
            for ci in range(CI):
                nc.sync.dma_start(out=hid_l[ci][:],
                                  in_=hid_t[ci * P:(ci + 1) * P, :])

            ones_b = singles.tile([P, P], bf16)
            nc.vector.memset(ones_b[:], 1.0)
            eps_col = singles.tile([P, 1], f32)
            nc.vector.memset(eps_col[:], EPS)

            bq_sb = None
            if flags["bq"]:
                bq_sb = singles.tile([P, CO], f32)
                nc.sync.dma_start(out=bq_sb[:],
                                  in_=bq_d[:].rearrange("(j p) -> p j", p=P))
            bk_sb = None
            if flags["bk"]:
                bk_sb = singles.tile([P, CO], f32)
                nc.sync.dma_start(out=bk_sb[:],
                                  in_=bk_d[:].rearrange("(j p) -> p j", p=P))
            bv_sb = None
            if flags["bv"]:
                bv_sb = singles.tile([1, C], bf16)
                nc.sync.dma_start(out=bv_sb[:], in_=bv_d[None, :])
            mb_sb = None
            if masked:
                mb_sb = singles.tile([P, ST], f32)
                nc.sync.dma_start(out=mb_sb[:],
                                  in_=mb_d[:].rearrange("(j p) -> p j", p=P))

            # ---------- QKV projections ----------
            QT = [singles.tile([P, T], bf16, tag=f"qt{i}", name=f"qt{i}")
                  for i in range(CO)]
            KTt = [singles.tile([P, T], bf16, tag=f"kt{i}", name=f"kt{i}")
                   for i in range(CO)]
            vaug = [singles.tile([P, H, DH + 1], bf16, tag=f"v{i}", name=f"v{i}")
                    for i in range(TT)]
            for tt in range(TT):
                nc.vector.memset(vaug[tt][:, :, DH:DH + 1], 1.0)

            for wdram, dest, bsb in ((wq, QT, bq_sb), (wk, KTt, bk_sb)):
                w_l = [wpool.tile([P, C], bf16, tag="w", name="w")
                       for _ in range(CI)]
                for ci in range(CI):
                    nc.sync.dma_start(out=w_l[ci][:],
                                      in_=wdram[ci * P:(ci + 1) * P, :])
                for cot in range(CO):
                    ps = ps_tile()
                    for ci in range(CI):
                        nc.tensor.matmul(ps[:],
                                         lhsT=w_l[ci][:, cot * P:(cot + 1) * P],
                                         rhs=hid_l[ci][:],
                                         start=(ci == 0), stop=(ci == CI - 1))
                    if bsb is not None:
                        nc.scalar.activation(dest[cot][:], ps[:], AF.Identity,
                                             bias=bsb[:, cot:cot + 1])
                    else:
                        nc.scalar.copy(dest[cot][:], ps[:])

            # rowsums of K_T per cotile (for the A-half of the mean)
            ksum_sb = None
            ksr = None
            if not masked:
                ksum_sb = singles.tile([P, CO], f32)
                ksr = [singles.tile([P, P], bf16, tag=f"ksr{i}", name=f"ksr{i}")
                       for i in range(CO)]
                for cot in range(CO):
                    nc.vector.tensor_reduce(ksum_sb[:, cot:cot + 1], KTt[cot][:],
                                            axis=AX.X, op=OP.add)
                    nc.vector.tensor_scalar_mul(ksr[cot][:], ones_b[:],
                                                ksum_sb[:, cot:cot + 1])

            NCH = 2
            CHW = C // NCH  # 384
            HPC = CHW // DH  # 6 heads per chunk
            w_l = [wpool.tile([P, C], bf16, tag="w", name="w")
                   for _ in range(CI)]
            for ci in range(CI):
                nc.sync.dma_start(out=w_l[ci][:], in_=wv[ci * P:(ci + 1) * P, :])
            for tt in range(TT):
                for ch in range(NCH):
                    vp = ps_tile()
                    for ci in range(CI):
                        nc.tensor.matmul(
                            vp[:, 0:CHW],
                            lhsT=hid_l[ci][:, tt * P:(tt + 1) * P],
                            rhs=w_l[ci][:, ch * CHW:(ch + 1) * CHW],
                            start=(ci == 0),
                            stop=(ci == CI - 1 and not flags["bv"]))
                    if flags["bv"]:
                        nc.tensor.matmul(vp[:, 0:CHW],
                                         lhsT=ones_b[0:1, :],
                                         rhs=bv_sb[:, ch * CHW:(ch + 1) * CHW],
                                         start=False, stop=True)
                    for hh in range(HPC):
                        nc.scalar.copy(vaug[tt][:, ch * HPC + hh, 0:DH],
                                       vp[:, hh * DH:(hh + 1) * DH])

            # ---------- late DMAs: dep maps + gating weights ----------
            dep_sb = singles.tile([P, ST, T], bf16)
            for j in range(ST):
                nc.sync.dma_start(out=dep_sb[:, j, :],
                                  in_=dep_t[j * P:(j + 1) * P, :])
            dpm_sb = singles.tile([P, ST, T], bf16)
            for j in range(ST):
                nc.sync.dma_start(out=dpm_sb[:, j, :],
                                  in_=dpm_t[j * P:(j + 1) * P, :])
            u_sb = None
            if not masked:
                u_sb = singles.tile([P, ST, T], bf16)
                for j in range(ST):
                    nc.sync.dma_start(out=u_sb[:, j, :],
                                      in_=u_t[j * P:(j + 1) * P, :])

            wg_sb = singles.tile([P, KT8, TM], bf16)
            for j in range(KT8):
                nc.sync.dma_start(out=wg_sb[:, j, :], in_=w1[j * P:(j + 1) * P, :])
            w2_sb = singles.tile([P, NT, TM], bf16)
            for j in range(NT):
                nc.sync.dma_start(out=w2_sb[:, j, :], in_=w2[j * P:(j + 1) * P, :])

            c_sb = None
            if flags["c"]:
                lnb_sb = singles.tile([P, KT8], bf16)
                nc.sync.dma_start(out=lnb_sb[:],
                                  in_=lnb_d[:].rearrange("(j p) -> p j", p=P))
                b1_sb = singles.tile([1, TM], f32)
                nc.sync.dma_start(out=b1_sb[:], in_=b1_d[None, :])
                c_ps = ps_tile()
                for j in range(KT8):
                    nc.tensor.matmul(c_ps[0:1, :], lhsT=lnb_sb[:, j:j + 1],
                                     rhs=wg_sb[:, j, :],
                                     start=(j == 0), stop=(j == KT8 - 1))
                c_sb = singles.tile([1, TM], f32r)
                nc.vector.tensor_add(c_sb[:], c_ps[0:1, :], b1_sb[:])

            if flags["lng"]:
                lng_sb = singles.tile([P, KT8], f32)
                nc.sync.dma_start(out=lng_sb[:],
                                  in_=lng_d[:].rearrange("(j p) -> p j", p=P))
                for j in range(KT8):
                    nc.vector.tensor_scalar_mul(wg_sb[:, j, :], wg_sb[:, j, :],
                                                lng_sb[:, j:j + 1])

            # s_vec = column sums of Wg (post ln_g fold); store negated f32r row
            s_ps = ps_tile()
            for j in range(KT8):
                nc.tensor.matmul(s_ps[0:1, :], lhsT=ones_b[:, 0:1],
                                 rhs=wg_sb[:, j, :],
                                 start=(j == 0), stop=(j == KT8 - 1))
            negs_sb = singles.tile([1, TM], f32r)
            nc.scalar.activation(negs_sb[:], s_ps[0:1, :], AF.Copy, scale=-1.0)

            b2_sb = None
            if flags["b2"]:
                b2_sb = singles.tile([1, TM], bf16)
                nc.sync.dma_start(out=b2_sb[:], in_=b2_d[None, :])

            # ---------- per-head pipeline ----------
            inv2t = 1.0 / (2 * TM)

            def scores_phase(h):
                """scores matmuls + A copy + D/diff'/s1 + squares for ms."""
                cot, poff = h // 2, (h % 2) * DH
                q_h = QT[cot][poff:poff + DH, :]
                k_h = KTt[cot][poff:poff + DH, :]
                A = [adpool.tile([P, T], bf16, tag="ad", name="A")
                     for _ in range(ST)]
                D = [adpool.tile([P, T], bf16, tag="ad", name="D")
                     for _ in range(ST)]
                dfs, s1s, wsqs = [], [], []
                for j in range(ST):
                    sp = ps_tile()
                    nc.tensor.matmul(sp[:], lhsT=k_h[:, j * P:(j + 1) * P],
                                     rhs=q_h, start=True, stop=True)
                    df = dfpool.tile([P, T], bf16, tag="df", name="df")
                    s1 = s1pool.tile([P, T], bf16, tag="s1", name="s1")
                    if masked:
                        nc.scalar.activation(A[j][:], sp[:], AF.Identity,
                                             bias=mb_sb[:, j:j + 1])
                        nc.vector.tensor_mul(D[j][:], sp[:], dep_sb[:, j, :])
                        # df = 0.5*(A - D) (mask bias lives in A only)
                        nc.vector.tensor_sub(df[:], A[j][:], D[j][:])
                        nc.vector.tensor_scalar_mul(df[:], df[:], 0.5)
                        nc.vector.tensor_add(s1[:], D[j][:], df[:])
                        sqa = sqpool.tile([P, T], bf16, tag="sq", name="sqa")
                        nc.vector.tensor_mul(sqa[:], A[j][:], A[j][:])
                        sqd = sqpool.tile([P, T], bf16, tag="sq", name="sqd")
                        nc.vector.tensor_mul(sqd[:], D[j][:], D[j][:])
                        wsqs.append((sqa, sqd))
                    else:
                        nc.scalar.copy(A[j][:], sp[:])
                        nc.vector.tensor_mul(D[j][:], A[j][:], dep_sb[:, j, :])
                        nc.vector.tensor_mul(df[:], A[j][:], dpm_sb[:, j, :])
                        nc.vector.tensor_add(s1[:], D[j][:], df[:])
                        sq = sqpool.tile([P, T], bf16, tag="sq", name="sq")
                        nc.vector.tensor_mul(sq[:], A[j][:], A[j][:])
                        wsq = sqpool.tile([P, T], bf16, tag="sq", name="wsq")
                        nc.vector.tensor_mul(wsq[:], sq[:], u_sb[:, j, :])
                        wsqs.append(wsq)
                    dfs.append(df)
                    s1s.append(s1)
                return A, D, dfs, s1s, wsqs

            def stats_mm_phase(h, A, D, wsqs):
                cot, poff = h // 2, (h % 2) * DH
                mu_ps = ps_tile()
                ms_ps = ps_tile()
                if masked:
                    for j, src in enumerate(A + D):
                        nc.tensor.matmul(mu_ps[:], lhsT=ones_b[:], rhs=src[:],
                                         start=(j == 0), stop=(j == 2 * ST - 1))
                    flat = [t for pair in wsqs for t in pair]
                    for j, src in enumerate(flat):
                        nc.tensor.matmul(ms_ps[:], lhsT=ones_b[:], rhs=src[:],
                                         start=(j == 0), stop=(j == len(flat) - 1))
                else:
                    nc.tensor.matmul(mu_ps[:],
                                     lhsT=ksr[cot][poff:poff + DH, :],
                                     rhs=QT[cot][poff:poff + DH, :],
                                     start=True, stop=False)
                    for j in range(ST):
                        nc.tensor.matmul(mu_ps[:], lhsT=ones_b[:], rhs=D[j][:],
                                         start=False, stop=(j == ST - 1))
                    for j, src in enumerate(wsqs):
                        nc.tensor.matmul(ms_ps[:], lhsT=ones_b[:], rhs=src[:],
                                         start=(j == 0), stop=(j == ST - 1))
                return mu_ps, ms_ps

            def stats_fin_phase(mu_ps, ms_ps):
                mu_rep = stpool.tile([P, T], f32r, tag="st", name="mu_rep")
                nc.scalar.activation(mu_rep[:], mu_ps[:], AF.Copy, scale=inv2t)
                mu2 = vrpool.tile([P, T], f32, tag="vr", name="mu2")
                nc.vector.tensor_mul(mu2[:], mu_rep[:], mu_rep[:])
                var = vrpool.tile([P, T], f32, tag="vr", name="var")
                nc.vector.scalar_tensor_tensor(var[:], ms_ps[:], inv2t, mu2[:],
                                               op0=OP.mult, op1=OP.subtract)
                pool = stpool if flags["c"] else vrpool
                tag = "st" if flags["c"] else "vr"
                sqv_rep = pool.tile([P, T], f32r, tag=tag, name="sqv_rep")
                nc.scalar.activation(sqv_rep[:], var[:], AF.Sqrt, bias=eps_col[:])
                rs_rep = stpool.tile([P, T], f32, tag="st", name="rs_rep")
                nc.vector.reciprocal(rs_rep[:], sqv_rep[:])
                return mu_rep, sqv_rep, rs_rep

            def mlp1_phase(A, D, mu_rep, sqv_rep, rs_rep):
                ti_l = []
                for nt in range(NT):
                    y_ps = ps_tile(psb)
                    nsl = slice(nt * P, (nt + 1) * P)
                    for j, src in enumerate(A + D):
                        nc.tensor.matmul(y_ps[:], lhsT=wg_sb[:, j, nsl],
                                         rhs=src[:], start=(j == 0), stop=False)
                    nc.tensor.matmul(y_ps[:], lhsT=negs_sb[0:1, nsl],
                                     rhs=mu_rep[0:1, :],
                                     start=False, stop=not flags["c"])
                    if flags["c"]:
                        nc.tensor.matmul(y_ps[:], lhsT=c_sb[0:1, nsl],
                                         rhs=sqv_rep[0:1, :],
                                         start=False, stop=True)
                    ti = tipool.tile([P, T], bf16, tag="ti", name="ti")
                    nc.vector.tensor_mul(ti[:], y_ps[:], rs_rep[:])
                    ti_l.append(ti)
                return ti_l

            def tanh_phase(ti_l):
                th_l = []
                for ti in ti_l:
                    th = thpool.tile([P, T], bf16, tag="th", name="th")
                    th_l.append(th)
                    nc.scalar.activation(th[:], ti[:], AF.Tanh)
                return th_l

            def mlp2_phase(th_l):
                g_ps_l = []
                for nt in range(NT):
                    g_ps = ps_tile(psb)
                    nsl = slice(nt * P, (nt + 1) * P)
                    for j in range(NT):
                        nc.tensor.matmul(g_ps[:], lhsT=w2_sb[:, j, nsl],
                                         rhs=th_l[j][:], start=(j == 0),
                                         stop=(j == NT - 1 and not flags["b2"]))
                    if flags["b2"]:
                        nc.tensor.matmul(g_ps[:], lhsT=b2_sb[0:1, nsl],
                                         rhs=ones_b[0:1, :],
                                         start=False, stop=True)
                    g_ps_l.append(g_ps)
                return g_ps_l

            def tanh2_phase(g_ps_l):
                # sigmoid(x) = 0.5 + 0.5*tanh(x/2); the affine part is folded
                # into the mix: mixd = s1 + t2*diff'
                t2_l = []
                for g_ps in g_ps_l:
                    t2 = t2pool.tile([P, T], bf16, tag="t2", name="t2")
                    t2_l.append(t2)
                    nc.scalar.activation(t2[:], g_ps[:], AF.Tanh, scale=0.5)
                return t2_l

            def mix_exp_phase(dfs, s1s, t2_l):
                E_l = []
                for j in range(ST):
                    prod = mixpool.tile([P, T], bf16, tag="mix", name="prod")
                    nc.gpsimd.tensor_mul(prod[:], t2_l[j][:], dfs[j][:])
                    mixd = mixpool.tile([P, T], bf16, tag="mix", name="mixd")
                    nc.gpsimd.tensor_add(mixd[:], prod[:], s1s[j][:])
                    E = epool.tile([P, T], bf16, tag="e", name="E")
                    nc.scalar.activation(E[:], mixd[:], AF.Exp)
                    E_l.append(E)
                return E_l

            def ctx_phase(h, E_l):
                # one psum bank: [ctx|den] chunks of width 65 at cols c*65
                W65 = DH + 1
                dc = ps_tile()
                for cch in range(4):
                    for j in range(ST):
                        nc.tensor.matmul(
                            dc[:, cch * W65:(cch + 1) * W65],
                            lhsT=E_l[j][:, cch * P:(cch + 1) * P],
                            rhs=vaug[j][:, h, :],
                            start=(j == 0), stop=(j == ST - 1))
                rden = rdpool.tile([P, 4], f32, tag="rd", name="rden")
                nc.vector.reciprocal(rden[:], dc[:, DH:4 * W65:W65])
                ctxn = opool.tile([P, 4, DH], f32, tag="o", name="ctxn")
                for cch in range(4):
                    nc.vector.tensor_scalar_mul(ctxn[:, cch, :],
                                                dc[:, cch * W65:cch * W65 + DH],
                                                rden[:, cch:cch + 1])
                nc.sync.dma_start(
                    out=out_t[:, h * DH:(h + 1) * DH].rearrange(
                        "(j p) c -> p j c", p=P),
                    in_=ctxn[:])

            # depth-2 software pipeline over head pairs
            NP = H // 2
            state = {}
            for pc in range(NP + 2):
                if pc < NP:
                    h0, h1 = 2 * pc, 2 * pc + 1
                    A0, D0, df0, s10, wsq0 = scores_phase(h0)
                    A1, D1, df1, s11, wsq1 = scores_phase(h1)
                    sm0 = stats_mm_phase(h0, A0, D0, wsq0)
                    sm1 = stats_mm_phase(h1, A1, D1, wsq1)
                    st0 = stats_fin_phase(*sm0)
                    st1 = stats_fin_phase(*sm1)
                if pc >= 2:
                    Ep = state[pc - 2]["E"]
                    ctx_phase(2 * (pc - 2), Ep[0])
                    ctx_phase(2 * (pc - 2) + 1, Ep[1])
                    del state[pc - 2]
                if 1 <= pc <= NP:
                    s = state[pc - 1]
                    ti0 = mlp1_phase(s["A0"], s["D0"], *s["st0"])
                    ti1 = mlp1_phase(s["A1"], s["D1"], *s["st1"])
                    th0 = tanh_phase(ti0)
                    th1 = tanh_phase(ti1)
                    gp0 = mlp2_phase(th0)
                    gp1 = mlp2_phase(th1)
                    t20 = tanh2_phase(gp0)
                    t21 = tanh2_phase(gp1)
                    e0 = mix_exp_phase(s["df0"], s["s10"], t20)
                    e1 = mix_exp_phase(s["df1"], s["s11"], t21)
                    s["E"] = (e0, e1)
                    # drop refs no longer needed
                    for k in ("A0", "A1", "D0", "D1", "df0", "df1",
                              "s10", "s11", "st0", "st1"):
                        s[k] = None
                if pc < NP:
                    state[pc] = dict(A0=A0, D0=D0, df0=df0, s10=s10, st0=st0,
                                     A1=A1, D1=D1, df1=df1, s11=s11, st1=st1)

    nc.compile()
    return nc


def _prep(inputs):
    bfloat16 = ml_dtypes.bfloat16
    hidden = np.asarray(inputs["hidden_states"], dtype=np.float32)
    mask = np.asarray(inputs["attention_mask"], dtype=np.float32)
    dep = np.asarray(inputs["dependency_matrix"], dtype=np.float32)
    ws = {k: np.asarray(inputs[k], dtype=np.float32)
          for k in ("Wq", "Wk", "Wv", "W1", "W2")}
    vs = {k: np.asarray(inputs[k], dtype=np.float32)
          for k in ("bq", "bk", "bv", "b1", "b2", "ln_g", "ln_b")}
    mb = (1.0 - mask) * -10000.0

    flags = {
        "bq": bool(np.any(vs["bq"])), "bk": bool(np.any(vs["bk"])),
        "bv": bool(np.any(vs["bv"])),
        "lng": bool(np.any(vs["ln_g"] != 1.0)),
        "c": bool(np.any(vs["ln_b"]) or np.any(vs["b1"])),
        "b2": bool(np.any(vs["b2"])),
        "mask": bool(np.any(mb)),
    }

    wq_b = np.ascontiguousarray((ws["Wq"] * np.float32(0.125)).astype(bfloat16))
    wk_b = np.ascontiguousarray(ws["Wk"].astype(bfloat16))
    wv_b = np.ascontiguousarray(ws["Wv"].astype(bfloat16))
    w1_b = np.ascontiguousarray(ws["W1"].astype(bfloat16))
    w2_b = np.ascontiguousarray(ws["W2"].astype(bfloat16))

    in_maps = []
    for b in range(N_CORES):
        dt = dep[b].T
        m = {
            "hid_t": np.ascontiguousarray(hidden[b].T.astype(bfloat16)),
            "dep_t": np.ascontiguousarray(dt.astype(bfloat16)),
            "dpm_t": np.ascontiguousarray((0.5 * (1.0 - dt)).astype(bfloat16)),
            "wq": wq_b, "wk": wk_b, "wv": wv_b,
            "w1": w1_b, "w2": w2_b,
        }
        if not flags["mask"]:
            m["u_t"] = np.ascontiguousarray((1.0 + dt * dt).astype(bfloat16))
        if flags["bq"]:
            m["bq"] = vs["bq"] * np.float32(0.125)
        if flags["bk"]:
            m["bk"] = vs["bk"]
        if flags["bv"]:
            m["bv"] = vs["bv"].astype(bfloat16)
        if flags["lng"]:
            m["lng"] = vs["ln_g"]
        if flags["c"]:
            m["lnb"] = vs["ln_b"].astype(bfloat16)
            m["b1"] = vs["b1"]
        if flags["b2"]:
            m["b2"] = vs["b2"].astype(bfloat16)
        if flags["mask"]:
            m["mb"] = np.ascontiguousarray(mb[b])
        in_maps.append(m)
    return flags, in_maps


def kernel(**inputs):
    from concourse.bass_utils import run_bass_kernel_spmd

    flags, in_maps = _prep(inputs)
    nc = _build(flags)
    res = run_bass_kernel_spmd(nc, in_maps, core_ids=list(range(N_CORES)))
    out = np.stack([r["out_t"] for r in res.results])
    return out.astype(np.float32)


# revision 13
# speedup vs baseline: 1.3583x; 1.1114x over previous
"""Trainium2 Bass kernel for nn_DependencyBertMix.

Contract: kernel(**inputs) takes the FULL unsharded inputs (as produced by
setup_inputs()) and returns the FULL [8, 512, 768] float32 output.

Strategy: data-parallel over batch - B=8 batch elements, one per NeuronCore.
Weights are replicated to all 8 cores; no collectives.

Per-core pipeline in transposed [feature, t] layout (t = query, s = key):

  Q'_T = (Wq/8)^T @ hid_T   (scores pre-scaled via Wq)   K_T, V likewise
  per head h:
    A_T[s,t]  = K_h as lhsT @ Q'_h          (= self_attn^T, bf16 copy)
    D_T       = A (.) dep^T                  (dep_self_attn^T)
    diff'     = A (.) 0.5(1-dep)^T           (= (A-D)/2)
    s1        = D + diff'                    (mix base)
    stats     : mu = (ksum_h @ Q'_h + colsum(D)) / 2T   (ksum = rowsum K_T)
                ms = colsum(A^2 (.) (1+dep^2)^T) = colsum(A^2+D^2)
                var/rsqrt on-chip, replicated rows via ones-matmuls
    LayerNorm + gating MLP folded into matmuls:
      Y[n,t]  = Wg^T @ [A;D] - s_vec x mu   (+ c_vec x sqrt(var+eps))
      th      = tanh(Y * rs)
      t2      = tanh(0.5 (W2^T th + b2))     (sigmoid(x) = 0.5+0.5 tanh(x/2))
    mixd      = s1 + t2 (.) diff'  =  g*A + (1-g)*D
    E = exp(mixd);  ctx[t,d] = E^T-chunks @ [V_h|1], * 1/den -> [T,C] output

Emission is a depth-2 software pipeline (iteration i runs scores/stats of
pair i, ctx of pair i-2, and the gating MLP of pair i-1) so the tensor
engine always has ready matmuls and PSUM slots recycle without stalls.
"""

import sys

for _p in ("/opt/trn_rl_repo", "/opt/pypackages"):
    if _p not in sys.path:
        sys.path.append(_p)

import ml_dtypes
import numpy as np

B, T, C = 8, 512, 768
H, DH = 12, 64
TM = 512
EPS = 1e-5
N_CORES = 8
P = 128


def _build(flags):
    import concourse.tile as tile
    from concourse import bacc, mybir

    f32 = mybir.dt.float32
    f32r = mybir.dt.float32r
    bf16 = mybir.dt.bfloat16
    AF = mybir.ActivationFunctionType
    OP = mybir.AluOpType
    AX = mybir.AxisListType

    nc = bacc.Bacc("TRN2", target_bir_lowering=False, debug=False,
                   enable_asserts=False, num_devices=N_CORES)

    masked = flags["mask"]

    # ---- DRAM I/O (host-prepared layouts; weights pre-cast to bf16) ----
    hid_t = nc.dram_tensor("hid_t", [C, T], bf16, kind="ExternalInput")
    dep_t = nc.dram_tensor("dep_t", [T, T], bf16, kind="ExternalInput")  # dep^T
    dpm_t = nc.dram_tensor("dpm_t", [T, T], bf16, kind="ExternalInput")  # (1-dep)^T/2
    if not masked:
        u_t = nc.dram_tensor("u_t", [T, T], bf16, kind="ExternalInput")  # 1+dep^2
    wq = nc.dram_tensor("wq", [C, C], bf16, kind="ExternalInput")  # pre /8
    wk = nc.dram_tensor("wk", [C, C], bf16, kind="ExternalInput")
    wv = nc.dram_tensor("wv", [C, C], bf16, kind="ExternalInput")
    w1 = nc.dram_tensor("w1", [2 * TM, TM], bf16, kind="ExternalInput")
    w2 = nc.dram_tensor("w2", [TM, TM], bf16, kind="ExternalInput")
    out_t = nc.dram_tensor("out_t", [T, C], f32, kind="ExternalOutput")

    bq_d = nc.dram_tensor("bq", [C], f32, kind="ExternalInput") if flags["bq"] else None
    bk_d = nc.dram_tensor("bk", [C], f32, kind="ExternalInput") if flags["bk"] else None
    bv_d = (nc.dram_tensor("bv", [C], bf16, kind="ExternalInput")
            if flags["bv"] else None)
    lng_d = (nc.dram_tensor("lng", [2 * TM], f32, kind="ExternalInput")
             if flags["lng"] else None)
    if flags["c"]:
        lnb_d = nc.dram_tensor("lnb", [2 * TM], bf16, kind="ExternalInput")
        b1_d = nc.dram_tensor("b1", [TM], f32, kind="ExternalInput")
    b2_d = (nc.dram_tensor("b2", [TM], bf16, kind="ExternalInput")
            if flags["b2"] else None)
    mb_d = (nc.dram_tensor("mb", [T], f32, kind="ExternalInput")
            if masked else None)

    CI = C // P   # 6
    CO = C // P   # 6
    ST = T // P   # 4
    TT = T // P   # 4
    KT8 = 2 * TM // P  # 8
    NT = TM // P  # 4

    with tile.TileContext(nc) as tc:
        with (
            tc.tile_pool(name="singles", bufs=1) as singles,
            tc.tile_pool(name="wpool", bufs=6) as wpool,
            tc.tile_pool(name="adpool", bufs=34) as adpool,
            tc.tile_pool(name="dfpool", bufs=17) as dfpool,
            tc.tile_pool(name="s1pool", bufs=17) as s1pool,
            tc.tile_pool(name="sqpool", bufs=10) as sqpool,
            tc.tile_pool(name="stpool", bufs=8) as stpool,
            tc.tile_pool(name="vrpool", bufs=6) as vrpool,
            tc.tile_pool(name="tipool", bufs=6) as tipool,
            tc.tile_pool(name="thpool", bufs=7) as thpool,
            tc.tile_pool(name="t2pool", bufs=7) as t2pool,
            tc.tile_pool(name="mixpool", bufs=6) as mixpool,
            tc.tile_pool(name="epool", bufs=15) as epool,
            tc.tile_pool(name="rdpool", bufs=4) as rdpool,
            tc.tile_pool(name="opool", bufs=4) as opool,
            tc.tile_pool(name="psf", bufs=4, space="PSUM") as psf,
            tc.tile_pool(name="psb", bufs=4, space="PSUM") as psb,
        ):
            def ps_tile(pool=None):
                pool = pool if pool is not None else psf
                return pool.tile([P, 512], f32, tag="ps", name="ps")

            # ---------- early DMAs: only what QKV needs ----------
            hid_l = [adpool.tile([P, T], bf16, tag="ad", name=f"hid{ci}")
                     for ci in range(CI)]
            for ci in range(CI):
                nc.sync.dma_start(out=hid_l[ci][:],
                                  in_=hid_t[ci * P:(ci + 1) * P, :])

            ones_b = singles.tile([P, P], bf16)
            nc.vector.memset(ones_b[:], 1.0)
            eps_col = singles.tile([P, 1], f32)
            nc.vector.memset(eps_col[:], EPS)

            bq_sb = None
            if flags["bq"]:
                bq_sb = singles.tile([P, CO], f32)
                nc.sync.dma_start(out=bq_sb[:],
                                  in_=bq_d[:].rearrange("(j p) -> p j", p=P))
            bk_sb = None
            if flags["bk"]:
                bk_sb = singles.tile([P, CO], f32)
                nc.sync.dma_start(out=bk_sb[:],
                                  in_=bk_d[:].rearrange("(j p) -> p j", p=P))
            bv_sb = None
            if flags["bv"]:
                bv_sb = singles.tile([1, C], bf16)
                nc.sync.dma_start(out=bv_sb[:], in_=bv_d[None, :])
            mb_sb = None
            if masked:
                mb_sb = singles.tile([P, ST], f32)
                nc.sync.dma_start(out=mb_sb[:],
                                  in_=mb_d[:].rearrange("(j p) -> p j", p=P))

            # ---------- QKV projections ----------
            QT = [singles.tile([P, T], bf16, tag=f"qt{i}", name=f"qt{i}")
                  for i in range(CO)]
            KTt = [singles.tile([P, T], bf16, tag=f"kt{i}", name=f"kt{i}")
                   for i in range(CO)]
            vaug = [singles.tile([P, H, DH + 1], bf16, tag=f"v{i}", name=f"v{i}")
                    for i in range(TT)]
            for tt in range(TT):
                nc.vector.memset(vaug[tt][:, :, DH:DH + 1], 1.0)

            for wdram, dest, bsb in ((wq, QT, bq_sb), (wk, KTt, bk_sb)):
                w_l = [wpool.tile([P, C], bf16, tag="w", name="w")
                       for _ in range(CI)]
                for ci in range(CI):
                    nc.sync.dma_start(out=w_l[ci][:],
                                      in_=wdram[ci * P:(ci + 1) * P, :])
                for cot in range(CO):
                    ps = ps_tile()
                    for ci in range(CI):
                        nc.tensor.matmul(ps[:],
                                         lhsT=w_l[ci][:, cot * P:(cot + 1) * P],
                                         rhs=hid_l[ci][:],
                                         start=(ci == 0), stop=(ci == CI - 1))
                    if bsb is not None:
                        nc.scalar.activation(dest[cot][:], ps[:], AF.Identity,
                                             bias=bsb[:, cot:cot + 1])
                    else:
                        nc.scalar.copy(dest[cot][:], ps[:])

            # rowsums of K_T per cotile (for the A-half of the mean)
            ksum_sb = None
            ksr = None
            if not masked:
                ksum_sb = singles.tile([P, CO], f32)
                ksr = [singles.tile([P, P], bf16, tag=f"ksr{i}", name=f"ksr{i}")
                       for i in range(CO)]
                for cot in range(CO):
                    nc.vector.tensor_reduce(ksum_sb[:, cot:cot + 1], KTt[cot][:],
                                            axis=AX.X, op=OP.add)
                    nc.vector.tensor_scalar_mul(ksr[cot][:], ones_b[:],
                                                ksum_sb[:, cot:cot + 1])

            NCH = 2
            CHW = C // NCH  # 384
            HPC = CHW // DH  # 6 heads per chunk
            w_l = [wpool.tile([P, C], bf16, tag="w", name="w")
                   for _ in range(CI)]
            for ci in range(CI):
                nc.sync.dma_start(out=w_l[ci][:], in_=wv[ci * P:(ci + 1) * P, :])
            for tt in range(TT):
                for ch in range(NCH):
                    vp = ps_tile()
                    for ci in range(CI):
                        nc.tensor.matmul(
                            vp[:, 0:CHW],
                            lhsT=hid_l[ci][:, tt * P:(tt + 1) * P],
                            rhs=w_l[ci][:, ch * CHW:(ch + 1) * CHW],
                            start=(ci == 0),
                            stop=(ci == CI - 1 and not flags["bv"]))
                    if flags["bv"]:
                        nc.tensor.matmul(vp[:, 0:CHW],
                                         lhsT=ones_b[0:1, :],
                                         rhs=bv_sb[:, ch * CHW:(ch + 1) * CHW],
                                         start=False, stop=True)
                    for hh in range(HPC):
                        nc.scalar.copy(vaug[tt][:, ch * HPC + hh, 0:DH],
                                       vp[:, hh * DH:(hh + 1) * DH])

            # ---------- late DMAs: dep maps + gating weights ----------
            dep_sb = singles.tile([P, ST, T], bf16)
            for j in range(ST):
                nc.sync.dma_start(out=dep_sb[:, j, :],
                                  in_=dep_t[j * P:(j + 1) * P, :])
            dpm_sb = singles.tile([P, ST, T], bf16)
            for j in range(ST):
                nc.sync.dma_start(out=dpm_sb[:, j, :],
                                  in_=dpm_t[j * P:(j + 1) * P, :])
            u_sb = None
            if not masked:
                u_sb = singles.tile([P, ST, T], bf16)
                for j in range(ST):
                    nc.sync.dma_start(out=u_sb[:, j, :],
                                      in_=u_t[j * P:(j + 1) * P, :])

            wg_sb = singles.tile([P, KT8, TM], bf16)
            for j in range(KT8):
                nc.sync.dma_start(out=wg_sb[:, j, :], in_=w1[j * P:(j + 1) * P, :])
            w2_sb = singles.tile([P, NT, TM], bf16)
            for j in range(NT):
                nc.sync.dma_start(out=w2_sb[:, j, :], in_=w2[j * P:(j + 1) * P, :])

            c_sb = None
            if flags["c"]:
                lnb_sb = singles.tile([P, KT8], bf16)
                nc.sync.dma_start(out=lnb_sb[:],
                                  in_=lnb_d[:].rearrange("(j p) -> p j", p=P))
                b1_sb = singles.tile([1, TM], f32)
                nc.sync.dma_start(out=b1_sb[:], in_=b1_d[None, :])
                c_ps = ps_tile()
                for j in range(KT8):
                    nc.tensor.matmul(c_ps[0:1, :], lhsT=lnb_sb[:, j:j + 1],
                                     rhs=wg_sb[:, j, :],
                                     start=(j == 0), stop=(j == KT8 - 1))
                c_sb = singles.tile([1, TM], f32r)
                nc.vector.tensor_add(c_sb[:], c_ps[0:1, :], b1_sb[:])

            if flags["lng"]:
                lng_sb = singles.tile([P, KT8], f32)
                nc.sync.dma_start(out=lng_sb[:],
                                  in_=lng_d[:].rearrange("(j p) -> p j", p=P))
                for j in range(KT8):
                    nc.vector.tensor_scalar_mul(wg_sb[:, j, :], wg_sb[:, j, :],
                                                lng_sb[:, j:j + 1])

            # s_vec = column sums of Wg (post ln_g fold); store negated f32r row
            s_ps = ps_tile()
            for j in range(KT8):
                nc.tensor.matmul(s_ps[0:1, :], lhsT=ones_b[:, 0:1],
                                 rhs=wg_sb[:, j, :],
                                 start=(j == 0), stop=(j == KT8 - 1))
            negs_sb = singles.tile([1, TM], f32r)
            nc.scalar.activation(negs_sb[:], s_ps[0:1, :], AF.Copy, scale=-1.0)

            b2_sb = None
            if flags["b2"]:
                b2_sb = singles.tile([1, TM], bf16)
                nc.sync.dma_start(out=b2_sb[:], in_=b2_d[None, :])

            # ---------- per-head pipeline ----------
            inv2t = 1.0 / (2 * TM)

            def scores_phase(h):
                """scores matmuls + A copy + D/diff'/s1 + squares for ms."""
                cot, poff = h // 2, (h % 2) * DH
                q_h = QT[cot][poff:poff + DH, :]
                k_h = KTt[cot][poff:poff + DH, :]
                A = [adpool.tile([P, T], bf16, tag="ad", name="A")
                     for _ in range(ST)]
                D = [adpool.tile([P, T], bf16, tag="ad", name="D")
                     for _ in range(ST)]
                dfs, s1s, wsqs = [], [], []
                for j in range(ST):
                    sp = ps_tile()
                    nc.tensor.matmul(sp[:], lhsT=k_h[:, j * P:(j + 1) * P],
                                     rhs=q_h, start=True, stop=True)
                    df = dfpool.tile([P, T], bf16, tag="df", name="df")
                    s1 = s1pool.tile([P, T], bf16, tag="s1", name="s1")
                    if masked:
                        nc.scalar.activation(A[j][:], sp[:], AF.Identity,
                                             bias=mb_sb[:, j:j + 1])
                        nc.vector.tensor_mul(D[j][:], sp[:], dep_sb[:, j, :])
                        # df = 0.5*(A - D) (mask bias lives in A only)
                        nc.vector.tensor_sub(df[:], A[j][:], D[j][:])
                        nc.vector.tensor_scalar_mul(df[:], df[:], 0.5)
                        nc.vector.tensor_add(s1[:], D[j][:], df[:])
                        sqa = sqpool.tile([P, T], bf16, tag="sq", name="sqa")
                        nc.gpsimd.tensor_mul(sqa[:], A[j][:], A[j][:])
                        sqd = sqpool.tile([P, T], bf16, tag="sq", name="sqd")
                        nc.gpsimd.tensor_mul(sqd[:], D[j][:], D[j][:])
                        wsqs.append((sqa, sqd))
                    else:
                        nc.scalar.copy(A[j][:], sp[:])
                        nc.vector.tensor_mul(D[j][:], A[j][:], dep_sb[:, j, :])
                        nc.vector.tensor_mul(df[:], A[j][:], dpm_sb[:, j, :])
                        nc.vector.tensor_add(s1[:], D[j][:], df[:])
                        sq = sqpool.tile([P, T], bf16, tag="sq", name="sq")
                        nc.gpsimd.tensor_mul(sq[:], A[j][:], A[j][:])
                        wsq = sqpool.tile([P, T], bf16, tag="sq", name="wsq")
                        nc.gpsimd.tensor_mul(wsq[:], sq[:], u_sb[:, j, :])
                        wsqs.append(wsq)
                    dfs.append(df)
                    s1s.append(s1)
                return A, D, dfs, s1s, wsqs

            def stats_mm_phase(h, A, D, wsqs):
                cot, poff = h // 2, (h % 2) * DH
                mu_ps = ps_tile()
                ms_ps = ps_tile()
                if masked:
                    for j, src in enumerate(A + D):
                        nc.tensor.matmul(mu_ps[:], lhsT=ones_b[:], rhs=src[:],
                                         start=(j == 0), stop=(j == 2 * ST - 1))
                    flat = [t for pair in wsqs for t in pair]
                    for j, src in enumerate(flat):
                        nc.tensor.matmul(ms_ps[:], lhsT=ones_b[:], rhs=src[:],
                                         start=(j == 0), stop=(j == len(flat) - 1))
                else:
                    nc.tensor.matmul(mu_ps[:],
                                     lhsT=ksr[cot][poff:poff + DH, :],
                                     rhs=QT[cot][poff:poff + DH, :],
                                     start=True, stop=False)
                    for j in range(ST):
                        nc.tensor.matmul(mu_ps[:], lhsT=ones_b[:], rhs=D[j][:],
                                         start=False, stop=(j == ST - 1))
                    for j, src in enumerate(wsqs):
                        nc.tensor.matmul(ms_ps[:], lhsT=ones_b[:], rhs=src[:],
                                         start=(j == 0), stop=(j == ST - 1))
                return mu_ps, ms_ps

            def stats_fin_phase(mu_ps, ms_ps):
                mu_rep = stpool.tile([P, T], f32r, tag="st", name="mu_rep")
                nc.scalar.activation(mu_rep[:], mu_ps[:], AF.Copy, scale=inv2t)
                mu2 = vrpool.tile([P, T], f32, tag="vr", name="mu2")
                nc.vector.tensor_mul(mu2[:], mu_rep[:], mu_rep[:])
                var = vrpool.tile([P, T], f32, tag="vr", name="var")
                nc.vector.scalar_tensor_tensor(var[:], ms_ps[:], inv2t, mu2[:],
                                               op0=OP.mult, op1=OP.subtract)
                pool = stpool if flags["c"] else vrpool
                tag = "st" if flags["c"] else "vr"
                sqv_rep = pool.tile([P, T], f32r, tag=tag, name="sqv_rep")
                nc.scalar.activation(sqv_rep[:], var[:], AF.Sqrt, bias=eps_col[:])
                rs_rep = stpool.tile([P, T], f32, tag="st", name="rs_rep")
                nc.vector.reciprocal(rs_rep[:], sqv_rep[:])
                return mu_rep, sqv_rep, rs_rep

            def mlp1_phase(A, D, mu_rep, sqv_rep, rs_rep):
                ti_l = []
                for nt in range(NT):
                    y_ps = ps_tile(psb)
                    nsl = slice(nt * P, (nt + 1) * P)
                    for j, src in enumerate(A + D):
                        nc.tensor.matmul(y_ps[:], lhsT=wg_sb[:, j, nsl],
                                         rhs=src[:], start=(j == 0), stop=False)
                    nc.tensor.matmul(y_ps[:], lhsT=negs_sb[0:1, nsl],
                                     rhs=mu_rep[0:1, :],
                                     start=False, stop=not flags["c"])
                    if flags["c"]:
                        nc.tensor.matmul(y_ps[:], lhsT=c_sb[0:1, nsl],
                                         rhs=sqv_rep[0:1, :],
                                         start=False, stop=True)
                    ti = tipool.tile([P, T], bf16, tag="ti", name="ti")
                    nc.vector.tensor_mul(ti[:], y_ps[:], rs_rep[:])
                    ti_l.append(ti)
                return ti_l

            def tanh_phase(ti_l):
                th_l = []
                for ti in ti_l:
                    th = thpool.tile([P, T], bf16, tag="th", name="th")
                    th_l.append(th)
                    nc.scalar.activation(th[:], ti[:], AF.Tanh)
                return th_l

            def mlp2_phase(th_l):
                g_ps_l = []
                for nt in range(NT):
                    g_ps = ps_tile(psb)
                    nsl = slice(nt * P, (nt + 1) * P)
                    for j in range(NT):
                        nc.tensor.matmul(g_ps[:], lhsT=w2_sb[:, j, nsl],
                                         rhs=th_l[j][:], start=(j == 0),
                                         stop=(j == NT - 1 and not flags["b2"]))
                    if flags["b2"]:
                        nc.tensor.matmul(g_ps[:], lhsT=b2_sb[0:1, nsl],
                                         rhs=ones_b[0:1, :],
                                         start=False, stop=True)
                    g_ps_l.append(g_ps)
                return g_ps_l

            def tanh2_phase(g_ps_l):
                # sigmoid(x) = 0.5 + 0.5*tanh(x/2); the affine part is folded
                # into the mix: mixd = s1 + t2*diff'
                t2_l = []
                for g_ps in g_ps_l:
                    t2 = t2pool.tile([P, T], bf16, tag="t2", name="t2")
                    t2_l.append(t2)
                    nc.scalar.activation(t2[:], g_ps[:], AF.Tanh, scale=0.5)
                return t2_l

            def mix_exp_phase(dfs, s1s, t2_l):
                E_l = []
                for j in range(ST):
                    prod = mixpool.tile([P, T], bf16, tag="mix", name="prod")
                    nc.vector.tensor_mul(prod[:], t2_l[j][:], dfs[j][:])
                    mixd = mixpool.tile([P, T], bf16, tag="mix", name="mixd")
                    nc.vector.tensor_add(mixd[:], prod[:], s1s[j][:])
                    E = epool.tile([P, T], bf16, tag="e", name="E")
                    nc.scalar.activation(E[:], mixd[:], AF.Exp)
                    E_l.append(E)
                return E_l

            def ctx_phase(h, E_l):
                # one psum bank: [ctx|den] chunks of width 65 at cols c*65
                W65 = DH + 1
                dc = ps_tile()
                for cch in range(4):
                    for j in range(ST):
                        nc.tensor.matmul(
                            dc[:, cch * W65:(cch + 1) * W65],
                            lhsT=E_l[j][:, cch * P:(cch + 1) * P],
                            rhs=vaug[j][:, h, :],
                            start=(j == 0), stop=(j == ST - 1))
                rden = rdpool.tile([P, 4], f32, tag="rd", name="rden")
                nc.vector.reciprocal(rden[:], dc[:, DH:4 * W65:W65])
                ctxn = opool.tile([P, 4, DH], f32, tag="o", name="ctxn")
                for cch in range(4):
                    nc.vector.tensor_scalar_mul(ctxn[:, cch, :],
                                                dc[:, cch * W65:cch * W65 + DH],
                                                rden[:, cch:cch + 1])
                nc.sync.dma_start(
                    out=out_t[:, h * DH:(h + 1) * DH].rearrange(
                        "(j p) c -> p j c", p=P),
                    in_=ctxn[:])

            # depth-2 software pipeline over head pairs
            NP = H // 2
            state = {}
            for pc in range(NP + 2):
                if pc < NP:
                    h0, h1 = 2 * pc, 2 * pc + 1
                    A0, D0, df0, s10, wsq0 = scores_phase(h0)
                    A1, D1, df1, s11, wsq1 = scores_phase(h1)
                    sm0 = stats_mm_phase(h0, A0, D0, wsq0)
                    sm1 = stats_mm_phase(h1, A1, D1, wsq1)
                    st0 = stats_fin_phase(*sm0)
                    st1 = stats_fin_phase(*sm1)
                if pc >= 2:
                    Ep = state[pc - 2]["E"]
                    ctx_phase(2 * (pc - 2), Ep[0])
                    ctx_phase(2 * (pc - 2) + 1, Ep[1])
                    del state[pc - 2]
                if 1 <= pc <= NP:
                    s = state[pc - 1]
                    ti0 = mlp1_phase(s["A0"], s["D0"], *s["st0"])
                    ti1 = mlp1_phase(s["A1"], s["D1"], *s["st1"])
                    th0 = tanh_phase(ti0)
                    th1 = tanh_phase(ti1)
                    gp0 = mlp2_phase(th0)
                    gp1 = mlp2_phase(th1)
                    t20 = tanh2_phase(gp0)
                    t21 = tanh2_phase(gp1)
                    e0 = mix_exp_phase(s["df0"], s["s10"], t20)
                    e1 = mix_exp_phase(s["df1"], s["s11"], t21)
                    s["E"] = (e0, e1)
                    # drop refs no longer needed
                    for k in ("A0", "A1", "D0", "D1", "df0", "df1",
                              "s10", "s11", "st0", "st1"):
                        s[k] = None
                if pc < NP:
                    state[pc] = dict(A0=A0, D0=D0, df0=df0, s10=s10, st0=st0,
                                     A1=A1, D1=D1, df1=df1, s11=s11, st1=st1)

    nc.compile()
    return nc


def _prep(inputs):
    bfloat16 = ml_dtypes.bfloat16
    hidden = np.asarray(inputs["hidden_states"], dtype=np.float32)
    mask = np.asarray(inputs["attention_mask"], dtype=np.float32)
    dep = np.asarray(inputs["dependency_matrix"], dtype=np.float32)
    ws = {k: np.asarray(inputs[k], dtype=np.float32)
          for k in ("Wq", "Wk", "Wv", "W1", "W2")}
    vs = {k: np.asarray(inputs[k], dtype=np.float32)
          for k in ("bq", "bk", "bv", "b1", "b2", "ln_g", "ln_b")}
    mb = (1.0 - mask) * -10000.0

    flags = {
        "bq": bool(np.any(vs["bq"])), "bk": bool(np.any(vs["bk"])),
        "bv": bool(np.any(vs["bv"])),
        "lng": bool(np.any(vs["ln_g"] != 1.0)),
        "c": bool(np.any(vs["ln_b"]) or np.any(vs["b1"])),
        "b2": bool(np.any(vs["b2"])),
        "mask": bool(np.any(mb)),
    }

    wq_b = np.ascontiguousarray((ws["Wq"] * np.float32(0.125)).astype(bfloat16))
    wk_b = np.ascontiguousarray(ws["Wk"].astype(bfloat16))
    wv_b = np.ascontiguousarray(ws["Wv"].astype(bfloat16))
    w1_b = np.ascontiguousarray(ws["W1"].astype(bfloat16))
    w2_b = np.ascontiguousarray(ws["W2"].astype(bfloat16))

    in_maps = []
    for b in range(N_CORES):
        dt = dep[b].T
        m = {
            "hid_t": np.ascontiguousarray(hidden[b].T.astype(bfloat16)),
            "dep_t": np.ascontiguousarray(dt.astype(bfloat16)),
            "dpm_t": np.ascontiguousarray((0.5 * (1.0 - dt)).astype(bfloat16)),
            "wq": wq_b, "wk": wk_b, "wv": wv_b,
            "w1": w1_b, "w2": w2_b,
        }
        if not flags["mask"]:
            m["u_t"] = np.ascontiguousarray((1.0 + dt * dt).astype(bfloat16))
        if flags["bq"]:
            m["bq"] = vs["bq"] * np.float32(0.125)
        if flags["bk"]:
            m["bk"] = vs["bk"]
        if flags["bv"]:
            m["bv"] = vs["bv"].astype(bfloat16)
        if flags["lng"]:
            m["lng"] = vs["ln_g"]
        if flags["c"]:
            m["lnb"] = vs["ln_b"].astype(bfloat16)
            m["b1"] = vs["b1"]
        if flags["b2"]:
            m["b2"] = vs["b2"].astype(bfloat16)
        if flags["mask"]:
            m["mb"] = np.ascontiguousarray(mb[b])
        in_maps.append(m)
    return flags, in_maps


def kernel(**inputs):
    from concourse.bass_utils import run_bass_kernel_spmd

    flags, in_maps = _prep(inputs)
    nc = _build(flags)
    res = run_bass_kernel_spmd(nc, in_maps, core_ids=list(range(N_CORES)))
    out = np.stack([r["out_t"] for r in res.results])
    return out.astype(np.float32)
